# revision 1
# baseline (speedup 1.0000x reference)
"""Trainium2 Bass kernel for a 2-layer LSTM + dense head (batch-sharded over 8 cores).

Reference computation (per PyTorch gate order i,f,g,o):
  h1 = LSTM(x;   w_ih1, w_hh1, b_ih1+b_hh1)   # D=128 -> H1=128
  h2 = LSTM(h1;  w_ih2, w_hh2, b_ih2+b_hh2)   # H1=128 -> H2=64
  out = relu(h2[:, -1] @ w_dense.T + b_dense) # [B, 64]

Device-side design (per core, B_c = 256 batch columns):
  - All state kept "transposed": hidden-dim on SBUF partitions, batch on free dim.
  - x is transposed on the host to [D, T, B_c] so every per-step rhs tile
    [D=128, B_c=256] is a contiguous DMA.
  - Gates are computed in PSUM via fp32r matmuls (1 cycle/row at N>=256):
      gates = W_ih^T.T @ x_t  +  bias (K=1 matmul)  +  W_hh^T.T @ h_{t-1}
  - ALL four gates go through a single tanh ACT op per layer:
      sigmoid(z) = (tanh(z/2)+1)/2 -- the 0.5 pre-scale for i,f,o is folded
      into the host-side weights/biases; the (t+1)/2 fix-ups are folded into
      scaled states: we store Ch = 2*c and Hh = 2*h. Compensation for Hh is
      folded into downstream weights (w_hh1, w_ih2, w_hh2, w_dense all /2).
    Per step (layer 1, tiles [128, 256]):
      t      = tanh(psum[i/2 | f/2 | o/2 | g])      # one ACT op [128,1024]
      u      = (t_i + 1) * t_g                      # = 2*sig_i*tanh_g
      v      = (t_f + 1) * Ch                       # = 4*sig_f*c
      Ch'    = 0.5*v + u                            # = 2*c'
      thc    = tanh(0.5 * Ch')                      # = tanh(c')
      Hh'    = (t_o + 1) * thc                      # = 2*h'
"""

import os
import numpy as np

import concourse.bass as bass
import concourse.mybir as mybir
from concourse import bacc
from concourse.tile import TileContext
from concourse.bass_utils import run_bass_kernel_spmd

N_CORES = 8
B, T, D = 2048, 128, 128
H1, H2, OUT = 128, 64, 64
BC = B // N_CORES  # 256 batch per core
TC = 32            # timesteps per x DMA chunk

FP = mybir.dt.float32
FPR = mybir.dt.float32r
AF = mybir.ActivationFunctionType
ALU = mybir.AluOpType

_PROGRAM_CACHE = {}


def _r(ap):
    """fp32r view of an fp32 AP (full-rate fp32 matmul at N>=256)."""
    return ap.bitcast(FPR)


def build_program():
    if "nc" in _PROGRAM_CACHE:
        return _PROGRAM_CACHE["nc"]

    nc = bacc.Bacc(
        "TRN2", target_bir_lowering=False, debug=False,
        enable_asserts=False, num_devices=N_CORES,
    )

    # ---- DRAM parameters (per-core shapes; in_maps supply per-core data)
    xT_d = nc.declare_dram_parameter("xT", [D, T, BC], FPR, isOutput=False)
    w1_d = nc.declare_dram_parameter("w1", [D, 4 * H1], FPR, isOutput=False)
    wh1_d = nc.declare_dram_parameter("wh1", [H1, 4 * H1], FPR, isOutput=False)
    b1_d = nc.declare_dram_parameter("b1", [1, 4 * H1], FPR, isOutput=False)
    w2_d = nc.declare_dram_parameter("w2", [H1, 4 * H2], FPR, isOutput=False)
    wh2_d = nc.declare_dram_parameter("wh2", [H2, 4 * H2], FPR, isOutput=False)
    b2_d = nc.declare_dram_parameter("b2", [1, 4 * H2], FPR, isOutput=False)
    wd_d = nc.declare_dram_parameter("wd", [H2, OUT], FPR, isOutput=False)
    bd_d = nc.declare_dram_parameter("bd", [1, OUT], FPR, isOutput=False)
    ones_d = nc.declare_dram_parameter("ones", [1, BC], FPR, isOutput=False)
    out_d = nc.declare_dram_parameter("outT", [OUT, BC], FP, isOutput=True)

    with TileContext(nc, num_cores=N_CORES) as tc:
        with (
            tc.tile_pool(name="const", bufs=1) as cpool,
            tc.tile_pool(name="xs", bufs=2) as xpool,
            tc.tile_pool(name="acts", bufs=2) as apool,
            tc.tile_pool(name="state", bufs=2) as spool,
            tc.tile_pool(name="ps1", bufs=2, space="PSUM") as ps1pool,
            tc.tile_pool(name="ps2", bufs=1, space="PSUM") as ps2pool,
            tc.tile_pool(name="psd", bufs=1, space="PSUM") as psdpool,
        ):
            # ---- load constants / weights into SBUF
            w1 = cpool.tile([D, 4 * H1], FPR, tag="w1")
            wh1 = cpool.tile([H1, 4 * H1], FPR, tag="wh1")
            b1 = cpool.tile([1, 4 * H1], FPR, tag="b1")
            w2 = cpool.tile([H1, 4 * H2], FPR, tag="w2")
            wh2 = cpool.tile([H2, 4 * H2], FPR, tag="wh2")
            b2 = cpool.tile([1, 4 * H2], FPR, tag="b2")
            wd = cpool.tile([H2, OUT], FPR, tag="wd")
            bd = cpool.tile([1, OUT], FPR, tag="bd")
            ones = cpool.tile([1, BC], FPR, tag="ones")
            for sb, dr in ((w1, w1_d), (wh1, wh1_d), (b1, b1_d), (w2, w2_d),
                           (wh2, wh2_d), (b2, b2_d), (wd, wd_d), (bd, bd_d),
                           (ones, ones_d)):
                nc.sync.dma_start(out=sb[:], in_=dr[:])

            # ---- initial cell states (Ch = 2c = 0); h states unused at t=0
            c1 = spool.tile([H1, BC], FP, tag="c1")
            c2 = spool.tile([H2, BC], FP, tag="c2")
            h1 = None
            h2 = None
            nc.vector.memset(c1[:], 0.0)
            nc.vector.memset(c2[:], 0.0)

            for kc in range(T // TC):
                xs = xpool.tile([D, TC, BC], FPR, tag="xs")
                nc.sync.dma_start(out=xs[:], in_=xT_d[:, kc * TC:(kc + 1) * TC, :])
                for tt in range(TC):
                    t = kc * TC + tt
                    xt = xs[:, tt, :]

                    # ================= layer 1 =================
                    # p1 spans 2 PSUM banks (quarters 0,1 / 2,3). Exactly one
                    # start=True per bank (2KB zero region); other first
                    # writers rely on lazy zero-region overwrite semantics.
                    p1 = ps1pool.tile([H1, 4, BC], FP, tag="p1")  # [i|f|o|g]
                    for j in (0, 2, 1, 3):
                        nc.tensor.matmul(p1[:, j, :], _r(w1[:, j * H1:(j + 1) * H1]),
                                         _r(xt), start=(j in (0, 2)), stop=False)
                    for j in range(4):
                        nc.tensor.matmul(p1[:, j, :], _r(b1[:, j * H1:(j + 1) * H1]),
                                         _r(ones[:]), start=False,
                                         stop=(t == 0 and j in (1, 3)))
                    if t > 0:
                        for j in range(4):
                            nc.tensor.matmul(p1[:, j, :], _r(wh1[:, j * H1:(j + 1) * H1]),
                                             _r(h1[:]), start=False, stop=(j in (1, 3)))

                    tg = apool.tile([H1, 4, BC], FP, tag="tg")
                    nc.scalar.activation(tg[:], p1[:], AF.Tanh)
                    t_i, t_f = tg[:, 0, :], tg[:, 1, :]
                    t_o, t_g = tg[:, 2, :], tg[:, 3, :]

                    u = apool.tile([H1, BC], FP, tag="u")
                    v = apool.tile([H1, BC], FP, tag="v")
                    c1n = spool.tile([H1, BC], FP, tag="c1")
                    thc = apool.tile([H1, BC], FP, tag="thc")
                    h1n = spool.tile([H1, BC], FPR, tag="h1")
                    nc.vector.scalar_tensor_tensor(u[:], t_i, 1.0, t_g, op0=ALU.add, op1=ALU.mult)
                    nc.vector.scalar_tensor_tensor(v[:], t_f, 1.0, c1[:], op0=ALU.add, op1=ALU.mult)
                    nc.vector.scalar_tensor_tensor(c1n[:], v[:], 0.5, u[:], op0=ALU.mult, op1=ALU.add)
                    nc.scalar.activation(thc[:], c1n[:], AF.Tanh, scale=0.5)
                    nc.vector.scalar_tensor_tensor(h1n[:], t_o, 1.0, thc[:], op0=ALU.add, op1=ALU.mult)
                    c1, h1 = c1n, h1n

                    # ================= layer 2 =================
                    # flat gate layout [H2=64 partitions, 4 gates, BC]:
                    # all elementwise operands share base partition 0.
                    # quarters 0,1 in bank0; 2,3 in bank1 of the 2-bank tile.
                    p2 = ps2pool.tile([H2, 4, BC], FP, tag="p2")
                    for m in (0, 2, 1, 3):
                        nc.tensor.matmul(p2[:, m, :], _r(w2[:, m * H2:(m + 1) * H2]),
                                         _r(h1[:]), start=(m in (0, 2)), stop=False)
                    for m in range(4):
                        nc.tensor.matmul(p2[:, m, :], _r(b2[:, m * H2:(m + 1) * H2]),
                                         _r(ones[:]), start=False,
                                         stop=(t == 0 and m in (1, 3)))
                    if t > 0:
                        for m in range(4):
                            nc.tensor.matmul(p2[:, m, :], _r(wh2[:, m * H2:(m + 1) * H2]),
                                             _r(h2[:]), start=False, stop=(m in (1, 3)))

                    t2 = apool.tile([H2, 4, BC], FP, tag="t2")
                    nc.scalar.activation(t2[:], p2[:], AF.Tanh)
                    t_i2, t_f2 = t2[:, 0, :], t2[:, 1, :]
                    t_o2, t_g2 = t2[:, 2, :], t2[:, 3, :]

                    u2 = apool.tile([H2, BC], FP, tag="u2")
                    v2 = apool.tile([H2, BC], FP, tag="v2")
                    c2n = spool.tile([H2, BC], FP, tag="c2")
                    thc2 = apool.tile([H2, BC], FP, tag="thc2")
                    h2n = spool.tile([H2, BC], FPR, tag="h2")
                    nc.vector.scalar_tensor_tensor(u2[:], t_i2, 1.0, t_g2, op0=ALU.add, op1=ALU.mult)
                    nc.vector.scalar_tensor_tensor(v2[:], t_f2, 1.0, c2[:], op0=ALU.add, op1=ALU.mult)
                    nc.vector.scalar_tensor_tensor(c2n[:], v2[:], 0.5, u2[:], op0=ALU.mult, op1=ALU.add)
                    nc.scalar.activation(thc2[:], c2n[:], AF.Tanh, scale=0.5)
                    nc.vector.scalar_tensor_tensor(h2n[:], t_o2, 1.0, thc2[:], op0=ALU.add, op1=ALU.mult)
                    c2, h2 = c2n, h2n

            # ---- dense head on h2[T-1]
            pd = psdpool.tile([OUT, BC], FP, tag="pd")
            nc.tensor.matmul(pd[:], _r(wd[:]), _r(h2[:]), start=True, stop=False)
            nc.tensor.matmul(pd[:], _r(bd[:]), _r(ones[:]), start=False, stop=True)
            outs = cpool.tile([OUT, BC], FP, tag="outs")
            nc.scalar.activation(outs[:], pd[:], AF.Relu)
            nc.sync.dma_start(out=out_d[:], in_=outs[:])

    nc.finalize()
    _PROGRAM_CACHE["nc"] = nc
    return nc


def _prep_inputs(x, w_ih1, w_hh1, b_ih1, b_hh1, w_ih2, w_hh2, b_ih2, b_hh2,
                 w_dense, b_dense):
    """Host-side layout + scaling prep. Gate order on device: [i, f, o, g].

    Scales: 0.5 pre-scale on i,f,o gate weights (sigmoid-via-tanh);
    h-carrying weights (w_hh1, w_ih2, w_hh2, w_dense) divided by 2 because
    stored hidden states are 2*h.
    """
    f32 = np.float32

    def gate_cols(w_t, H, pre):  # w_t: [in_dim, 4H] in (i,f,g,o) order -> (i,f,o,g)*pre
        i, f, g, o = (w_t[:, k * H:(k + 1) * H] for k in range(4))
        return np.concatenate([0.5 * i, 0.5 * f, 0.5 * o, g], axis=1).astype(f32) * pre

    def gate_vec(bv, H):
        i, f, g, o = (bv[k * H:(k + 1) * H] for k in range(4))
        return np.concatenate([0.5 * i, 0.5 * f, 0.5 * o, g]).astype(f32)[None, :]

    w1 = gate_cols(np.ascontiguousarray(w_ih1.T), H1, 1.0)
    wh1 = gate_cols(np.ascontiguousarray(w_hh1.T), H1, 0.5)
    b1 = gate_vec(b_ih1 + b_hh1, H1)
    w2 = gate_cols(np.ascontiguousarray(w_ih2.T), H2, 0.5)
    wh2 = gate_cols(np.ascontiguousarray(w_hh2.T), H2, 0.5)
    b2 = gate_vec(b_ih2 + b_hh2, H2)
    wd = (np.ascontiguousarray(w_dense.T) * 0.5).astype(f32)
    bd = b_dense.astype(f32)[None, :]

    xT = np.ascontiguousarray(np.asarray(x, dtype=f32).transpose(2, 1, 0))  # [D,T,B]
    shared = dict(w1=w1, wh1=wh1, b1=b1, w2=w2, wh2=wh2, b2=b2, wd=wd, bd=bd,
                  ones=np.ones((1, BC), f32))
    in_maps = []
    for c in range(N_CORES):
        m = dict(shared)
        m["xT"] = np.ascontiguousarray(xT[:, :, c * BC:(c + 1) * BC])
        in_maps.append(m)
    return in_maps


def _run(inputs, trace=False, **kw):
    nc = build_program()
    in_maps = _prep_inputs(**inputs)
    res = run_bass_kernel_spmd(nc, in_maps, list(range(N_CORES)), trace=trace, **kw)
    out = np.concatenate([np.asarray(res.results[c]["outT"]).T for c in range(N_CORES)], axis=0)
    return out.astype(np.float32), res


def kernel(**inputs):
    out, _ = _run(inputs, trace=False)
    return out


if __name__ == "__main__":
    import reference
    inputs = {k: np.asarray(v) for k, v in reference.setup_inputs().items()}
    expected = np.asarray(reference.reference(**inputs))
    out, res = _run(inputs, trace=os.environ.get("KTRACE", "0") == "1")
    err = np.abs(out - expected)
    rel = err.max() / (np.abs(expected).max() + 1e-12)
    print("max abs err:", err.max(), "rel:", rel)
    print("exec_time_ns:", res.exec_time_ns)



# revision 9
# speedup vs baseline: 1.7888x; 1.7888x over previous
"""Trainium2 Bass kernel for a 2-layer LSTM + dense head (batch-sharded over 8 cores).

Reference computation (per PyTorch gate order i,f,g,o):
  h1 = LSTM(x;   w_ih1, w_hh1, b_ih1+b_hh1)   # D=128 -> H1=128
  h2 = LSTM(h1;  w_ih2, w_hh2, b_ih2+b_hh2)   # H1=128 -> H2=64
  out = relu(h2[:, -1] @ w_dense.T + b_dense) # [B, 64]

Device-side design (per core, B_c = 256 batch columns):
  - All state "transposed": hidden on partitions, batch on free dim.
  - Layer 1 minimizes the recurrent chain: gate preacts in three PSUM groups
    [i,f] / [g] / [o]; sigmoid/tanh applied directly (tanh+sigmoid+relu share
    one activation table set), outputs in bf16 so the whole cell update runs
    as four tensor_tensor ops at 2x DVE rate:
      v   = s_f * c         (bf16 TT)
      u   = s_i * t_g       (bf16 TT)
      c'  = v + u           (bf16 TT)
      thc = tanh(c')        (ACT)
      h'  = s_o * thc       (bf16 TT)
    W_ih/bias matmuls run ahead; only W_hh ([i,f] then [g]) gates the chain.
    The L1 step outranks the previous step's L2 block in the Tile scheduler.
  - Layer 2 (H2=64) is batch-packed: partitions = (batch-half, h2), free =
    128 batch cols, halving every L2 ACT/DVE op. Its matmuls use zero-padded
    bf16 stationaries (bf16 runs 1 cycle/row at any width; fp32r needs >=256
    cols). L2 uses the tanh-only trick (sigmoid(z) = (tanh(z/2)+1)/2 with
    0.5-prescaled weights; state Ch2 = 2*c2, Hh2 = 2*h2; wh2/wd are halved):
    one big 4-gate tanh + STT chain, which is cheaper on ACT and has a full
    step of slack behind L1.
"""

import os
import numpy as np

import concourse.bass as bass
import concourse.mybir as mybir
from concourse import bacc
from concourse.tile import TileContext
from concourse.bass_utils import run_bass_kernel_spmd

N_CORES = 8
B, T, D = 2048, 128, 128
H1, H2, OUT = 128, 64, 64
BC = B // N_CORES  # 256 batch per core
BH = BC // 2       # 128 = batch half (L2 packed free dim)
TC = 32            # timesteps per x DMA chunk

FP = mybir.dt.float32
FPR = mybir.dt.float32r
BF = mybir.dt.bfloat16
AF = mybir.ActivationFunctionType
ALU = mybir.AluOpType

_PROGRAM_CACHE = {}


def _r(ap):
    """fp32r view of an fp32 AP (full-rate fp32 matmul at N>=256)."""
    return ap.bitcast(FPR)


def build_program():
    if "nc" in _PROGRAM_CACHE:
        return _PROGRAM_CACHE["nc"]

    nc = bacc.Bacc(
        "TRN2", target_bir_lowering=False, debug=False,
        enable_asserts=False, num_devices=N_CORES,
    )

    # ---- DRAM parameters (per-core shapes; in_maps supply per-core data)
    xT_d = nc.declare_dram_parameter("xT", [D, T, BC], FPR, isOutput=False)
    # L1: gate groups [i, f] / [g] / [o], unscaled weights.
    wif_d = nc.declare_dram_parameter("wif", [D, 2 * H1], FPR, isOutput=False)
    wg_d = nc.declare_dram_parameter("wg", [D, H1], FPR, isOutput=False)
    wo_d = nc.declare_dram_parameter("wo", [D, H1], FPR, isOutput=False)
    whif_d = nc.declare_dram_parameter("whif", [H1, 2 * H1], BF, isOutput=False)
    whg_d = nc.declare_dram_parameter("whg", [H1, H1], BF, isOutput=False)
    who_d = nc.declare_dram_parameter("who", [H1, H1], BF, isOutput=False)
    bif_d = nc.declare_dram_parameter("bif", [1, 2 * H1], FPR, isOutput=False)
    bg_d = nc.declare_dram_parameter("bg", [1, H1], FPR, isOutput=False)
    bo_d = nc.declare_dram_parameter("bo", [1, H1], FPR, isOutput=False)
    # L2 packed stationaries (zero-padded, bf16). Gate order [i, g, f, o],
    # 0.5-prescaled on i/f/o for the tanh trick.
    w2p_d = nc.declare_dram_parameter("w2p", [H1, 8, 2 * H2], BF, isOutput=False)
    wh2p_d = nc.declare_dram_parameter("wh2p", [2 * H2, 8, 2 * H2], BF, isOutput=False)
    b2p_d = nc.declare_dram_parameter("b2p", [1, 4, 2 * H2], BF, isOutput=False)
    wdp_d = nc.declare_dram_parameter("wdp", [2 * H2, 2, OUT], BF, isOutput=False)
    bd_d = nc.declare_dram_parameter("bd", [1, OUT], FPR, isOutput=False)
    ones_d = nc.declare_dram_parameter("ones", [1, BC], FPR, isOutput=False)
    onesb_d = nc.declare_dram_parameter("onesb", [1, BC], BF, isOutput=False)
    out_d = nc.declare_dram_parameter("outT", [OUT, BC], FP, isOutput=True)

    with TileContext(nc, num_cores=N_CORES) as tc:
        with (
            tc.tile_pool(name="const", bufs=1) as cpool,
            tc.tile_pool(name="xs", bufs=2) as xpool,
            tc.tile_pool(name="acts", bufs=2) as apool,
            tc.tile_pool(name="state", bufs=2) as spool,
            tc.tile_pool(name="psIF", bufs=2, space="PSUM") as psIFpool,  # [H1,2,BC] 1 bank
            tc.tile_pool(name="psG", bufs=2, space="PSUM") as psGpool,    # [H1,BC]
            tc.tile_pool(name="psO", bufs=2, space="PSUM") as psOpool,    # [H1,BC]
            tc.tile_pool(name="ps2", bufs=2, space="PSUM") as ps2pool,    # [128,4,BH] 1 bank
        ):
            # ---- load constants / weights into SBUF
            wif = cpool.tile([D, 2 * H1], FPR, tag="wif")
            wg = cpool.tile([D, H1], FPR, tag="wg")
            wo = cpool.tile([D, H1], FPR, tag="wo")
            whif = cpool.tile([H1, 2 * H1], BF, tag="whif")
            whg = cpool.tile([H1, H1], BF, tag="whg")
            who = cpool.tile([H1, H1], BF, tag="who")
            bif = cpool.tile([1, 2 * H1], FPR, tag="bif")
            bg = cpool.tile([1, H1], FPR, tag="bg")
            bo = cpool.tile([1, H1], FPR, tag="bo")
            w2p = cpool.tile([H1, 8, 2 * H2], BF, tag="w2p")
            wh2p = cpool.tile([2 * H2, 8, 2 * H2], BF, tag="wh2p")
            b2p = cpool.tile([1, 4, 2 * H2], BF, tag="b2p")
            wdp = cpool.tile([2 * H2, 2, OUT], BF, tag="wdp")
            bd = cpool.tile([1, OUT], FPR, tag="bd")
            ones = cpool.tile([1, BC], FPR, tag="ones")
            onesb = cpool.tile([1, BC], BF, tag="onesb")
            for sb, dr in ((wif, wif_d), (wg, wg_d), (wo, wo_d), (whif, whif_d),
                           (whg, whg_d), (who, who_d), (bif, bif_d), (bg, bg_d),
                           (bo, bo_d), (w2p, w2p_d), (wh2p, wh2p_d), (b2p, b2p_d),
                           (wdp, wdp_d), (bd, bd_d), (ones, ones_d), (onesb, onesb_d)):
                nc.sync.dma_start(out=sb[:], in_=dr[:])

            # ---- initial states: c1 (true c, bf16), c2 (Ch2 = 2*c2, fp32)
            c1 = spool.tile([H1, BC], BF, tag="c1")
            c2 = spool.tile([2 * H2, BH], FP, tag="c2")
            h1 = None
            h2 = None
            nc.vector.memset(c1[:], 0.0)
            nc.vector.memset(c2[:], 0.0)

            for kc in range(T // TC):
                xs = xpool.tile([D, TC, BC], FPR, tag="xs")
                nc.sync.dma_start(out=xs[:], in_=xT_d[:, kc * TC:(kc + 1) * TC, :])
                for tt in range(TC):
                    t = kc * TC + tt
                    xt = xs[:, tt, :]

                    # ================= layer 1 =================
                    ctx_hp = tc.high_priority(offset=60)
                    ctx_hp.__enter__()
                    pIF = psIFpool.tile([H1, 2, BC], FP, tag="pIF")
                    pG = psGpool.tile([H1, BC], FP, tag="pG")
                    pO = psOpool.tile([H1, BC], FP, tag="pO")
                    nc.tensor.matmul(pIF[:, 0, :], _r(wif[:, 0:H1]), _r(xt),
                                     start=True, stop=False)
                    nc.tensor.matmul(pIF[:, 1, :], _r(wif[:, H1:2 * H1]), _r(xt),
                                     start=False, stop=False)
                    nc.tensor.matmul(pG[:], _r(wg[:]), _r(xt), start=True, stop=False)
                    nc.tensor.matmul(pO[:], _r(wo[:]), _r(xt), start=True, stop=False)
                    nc.tensor.matmul(pIF[:, 0, :], _r(bif[:, 0:H1]), _r(ones[:]),
                                     start=False, stop=False)
                    nc.tensor.matmul(pIF[:, 1, :], _r(bif[:, H1:2 * H1]), _r(ones[:]),
                                     start=False, stop=(t == 0))
                    nc.tensor.matmul(pG[:], _r(bg[:]), _r(ones[:]),
                                     start=False, stop=(t == 0))
                    nc.tensor.matmul(pO[:], _r(bo[:]), _r(ones[:]),
                                     start=False, stop=(t == 0))
                    if t > 0:
                        # W_hh in batch-half columns (bf16 runs full rate at
                        # N=128): the first half's matmuls start while the
                        # second h1 half is still being computed.
                        for lo, hi, last in ((0, BH, False), (BH, BC, True)):
                            nc.tensor.matmul(pIF[:, 0, lo:hi], whif[:, 0:H1],
                                             h1[:, lo:hi], start=False, stop=False)
                            nc.tensor.matmul(pIF[:, 1, lo:hi], whif[:, H1:2 * H1],
                                             h1[:, lo:hi], start=False, stop=last)
                            nc.tensor.matmul(pG[:, lo:hi], whg[:], h1[:, lo:hi],
                                             start=False, stop=last)
                            nc.tensor.matmul(pO[:, lo:hi], who[:], h1[:, lo:hi],
                                             start=False, stop=last)

                    s_if = apool.tile([H1, 2, BC], BF, tag="s_if")
                    nc.scalar.activation(s_if[:], pIF[:], AF.Sigmoid)
                    t_g = apool.tile([H1, BC], BF, tag="t_g")
                    nc.scalar.activation(t_g[:], pG[:], AF.Tanh)
                    s_o = apool.tile([H1, BC], BF, tag="s_o")
                    nc.scalar.activation(s_o[:], pO[:], AF.Sigmoid)

                    v = apool.tile([H1, BC], BF, tag="v")
                    u = apool.tile([H1, BC], BF, tag="u")
                    c1n = spool.tile([H1, BC], BF, tag="c1")
                    thc = apool.tile([H1, BC], BF, tag="thc")
                    h1n = spool.tile([H1, BC], BF, tag="h1")
                    nc.vector.tensor_tensor(v[:], s_if[:, 1, :], c1[:], op=ALU.mult)
                    nc.vector.tensor_tensor(u[:], s_if[:, 0, :], t_g[:], op=ALU.mult)
                    nc.vector.tensor_tensor(c1n[:], v[:], u[:], op=ALU.add)
                    nc.scalar.activation(thc[:], c1n[:], AF.Tanh)
                    # h1n in batch halves so the first half's W_hh matmuls
                    # overlap the second half's compute.
                    nc.vector.tensor_tensor(h1n[:, 0:BH], s_o[:, 0:BH],
                                            thc[:, 0:BH], op=ALU.mult)
                    nc.vector.tensor_tensor(h1n[:, BH:BC], s_o[:, BH:BC],
                                            thc[:, BH:BC], op=ALU.mult)
                    c1, h1 = c1n, h1n
                    ctx_hp.__exit__(None, None, None)

                    # ================= layer 2 (batch-packed) =================
                    # p2 [128=(bh,h2), 4, BH]: one PSUM bank; gate order
                    # [i, g, f, o]. Zero-padded bf16 stationaries, N=BH each;
                    # the inactive partition half accumulates zeros.
                    p2 = ps2pool.tile([2 * H2, 4, BH], FP, tag="p2")
                    for m in range(4):
                        nc.tensor.matmul(p2[:, m, :], b2p[:, m, :], onesb[:, 0:BH],
                                         start=(m == 0), stop=False)
                    for m in range(4):
                        nc.tensor.matmul(p2[:, m, :], w2p[:, 2 * m, :], h1[:, 0:BH],
                                         start=False, stop=False)
                        nc.tensor.matmul(p2[:, m, :], w2p[:, 2 * m + 1, :], h1[:, BH:BC],
                                         start=False, stop=(t == 0 and m == 3))
                    if t > 0:
                        for m in range(4):
                            nc.tensor.matmul(p2[:, m, :], wh2p[:, 2 * m, :], h2[:],
                                             start=False, stop=False)
                            nc.tensor.matmul(p2[:, m, :], wh2p[:, 2 * m + 1, :], h2[:],
                                             start=False, stop=(m == 3))

                    t2 = apool.tile([2 * H2, 4, BH], FP, tag="t2")
                    nc.scalar.activation(t2[:], p2[:], AF.Tanh)
                    t_i2, t_g2 = t2[:, 0, :], t2[:, 1, :]
                    t_f2, t_o2 = t2[:, 2, :], t2[:, 3, :]

                    u2 = apool.tile([2 * H2, BH], FP, tag="u2")
                    v2 = apool.tile([2 * H2, BH], FP, tag="v2")
                    c2n = spool.tile([2 * H2, BH], FP, tag="c2")
                    thc2 = apool.tile([2 * H2, BH], FP, tag="thc2")
                    h2n = spool.tile([2 * H2, BH], BF, tag="h2")
                    nc.vector.scalar_tensor_tensor(u2[:], t_i2, 1.0, t_g2, op0=ALU.add, op1=ALU.mult)
                    nc.vector.scalar_tensor_tensor(v2[:], t_f2, 1.0, c2[:], op0=ALU.add, op1=ALU.mult)
                    nc.vector.scalar_tensor_tensor(c2n[:], v2[:], 0.5, u2[:], op0=ALU.mult, op1=ALU.add)
                    nc.scalar.activation(thc2[:], c2n[:], AF.Tanh, scale=0.5)
                    nc.vector.scalar_tensor_tensor(h2n[:], t_o2, 1.0, thc2[:], op0=ALU.add, op1=ALU.mult)
                    c2, h2 = c2n, h2n

            # ---- dense head on h2[T-1] (packed): block A -> out cols 0:BH,
            # block B -> out cols BH:BC. Reuses the pO PSUM tag.
            pdt = psOpool.tile([H1, BC], FP, tag="pO")
            pd = pdt[0:OUT, :]
            nc.tensor.matmul(pd[:, 0:BH], wdp[:, 0, :], h2[:], start=True, stop=False)
            nc.tensor.matmul(pd[:, BH:BC], wdp[:, 1, :], h2[:], start=False, stop=False)
            nc.tensor.matmul(pd, _r(bd[:]), _r(ones[:]), start=False, stop=True)
            outs = cpool.tile([OUT, BC], FP, tag="outs")
            nc.scalar.activation(outs[:], pd, AF.Relu)
            nc.sync.dma_start(out=out_d[:], in_=outs[:])

    nc.finalize()
    _PROGRAM_CACHE["nc"] = nc
    return nc


def _prep_inputs(x, w_ih1, w_hh1, b_ih1, b_hh1, w_ih2, w_hh2, b_ih2, b_hh2,
                 w_dense, b_dense):
    """Host-side layout prep.

    L1: unscaled, gate groups [i,f] / [g] / [o] (device applies sigmoid/tanh
    directly; h1/c1 are true-valued).
    L2: packed padded [i,g,f,o] with the tanh trick: 0.5 pre-scale on i/f/o;
    wh2 and w_dense additionally halved because stored L2 states are 2*h2.
    """
    f32 = np.float32
    bf16 = mybir.dt.np(BF)

    def torch_gates(w_t, H):
        # [in_dim, 4H] torch (i,f,g,o) order -> dict of per-gate blocks
        i, f, g, o = (w_t[:, k * H:(k + 1) * H] for k in range(4))
        return i, f, g, o

    # ---- L1 (unscaled)
    i1, f1, g1, o1 = torch_gates(np.ascontiguousarray(w_ih1.T), H1)
    hi1, hf1, hg1, ho1 = torch_gates(np.ascontiguousarray(w_hh1.T), H1)
    bi1, bf1_, bg1, bo1 = (np.asarray(b_ih1 + b_hh1)[k * H1:(k + 1) * H1] for k in range(4))
    wif = np.concatenate([i1, f1], axis=1).astype(f32)
    wg = g1.astype(f32)
    wo = o1.astype(f32)
    whif = np.concatenate([hi1, hf1], axis=1).astype(bf16)
    whg = hg1.astype(bf16)
    who = ho1.astype(bf16)
    bifv = np.concatenate([bi1, bf1_]).astype(f32)[None, :]
    bgv = bg1.astype(f32)[None, :]
    bov = bo1.astype(f32)[None, :]

    # ---- L2 packed (partitions = (batch-half, h2)), tanh trick
    def gates2(w_t, H, pre):
        i, f, g, o = torch_gates(w_t, H)
        return [a * pre for a in (0.5 * i, g, 0.5 * f, 0.5 * o)]

    g2 = gates2(np.ascontiguousarray(w_ih2.T), H2, 1.0)      # consumes true h1
    gh2 = gates2(np.ascontiguousarray(w_hh2.T), H2, 0.5)     # consumes Hh2 = 2*h2
    bsum2 = np.asarray(b_ih2 + b_hh2)
    gb2 = [0.5 * bsum2[0:H2], bsum2[2 * H2:3 * H2], 0.5 * bsum2[H2:2 * H2],
           0.5 * bsum2[3 * H2:4 * H2]]
    w2p = np.zeros((H1, 8, 2 * H2), np.float32)
    wh2p = np.zeros((2 * H2, 8, 2 * H2), np.float32)
    b2p = np.zeros((1, 4, 2 * H2), np.float32)
    for m in range(4):
        w2p[:, 2 * m, 0:H2] = g2[m]           # block A -> partitions 0:64
        w2p[:, 2 * m + 1, H2:2 * H2] = g2[m]  # block B -> partitions 64:128
        wh2p[0:H2, 2 * m, 0:H2] = gh2[m]
        wh2p[H2:2 * H2, 2 * m + 1, H2:2 * H2] = gh2[m]
        b2p[0, m, 0:H2] = gb2[m]
        b2p[0, m, H2:2 * H2] = gb2[m]
    wdt = np.ascontiguousarray(w_dense.T) * 0.5              # [H2, OUT]
    wdp = np.zeros((2 * H2, 2, OUT), np.float32)
    wdp[0:H2, 0, :] = wdt
    wdp[H2:2 * H2, 1, :] = wdt
    bd = np.asarray(b_dense).astype(f32)[None, :]

    xT = np.ascontiguousarray(np.asarray(x, dtype=f32).transpose(2, 1, 0))  # [D,T,B]
    shared = dict(wif=wif, wg=wg, wo=wo, whif=whif, whg=whg, who=who,
                  bif=bifv, bg=bgv, bo=bov,
                  w2p=w2p.astype(bf16), wh2p=wh2p.astype(bf16),
                  b2p=b2p.astype(bf16), wdp=wdp.astype(bf16), bd=bd,
                  ones=np.ones((1, BC), f32), onesb=np.ones((1, BC), bf16))
    in_maps = []
    for c in range(N_CORES):
        m = dict(shared)
        m["xT"] = np.ascontiguousarray(xT[:, :, c * BC:(c + 1) * BC])
        in_maps.append(m)
    return in_maps


def _unpack_out(res):
    """Per-core outT is [OUT, BC]; batch cols already in original order."""
    return np.concatenate(
        [np.asarray(res.results[c]["outT"]).T for c in range(N_CORES)], axis=0)


def _run(inputs, trace=False, **kw):
    nc = build_program()
    in_maps = _prep_inputs(**inputs)
    res = run_bass_kernel_spmd(nc, in_maps, list(range(N_CORES)), trace=trace, **kw)
    return _unpack_out(res).astype(np.float32), res


def kernel(**inputs):
    out, _ = _run(inputs, trace=False)
    return out


if __name__ == "__main__":
    import reference
    inputs = {k: np.asarray(v) for k, v in reference.setup_inputs().items()}
    expected = np.asarray(reference.reference(**inputs))
    out, res = _run(inputs, trace=os.environ.get("KTRACE", "0") == "1")
    err = np.abs(out - expected)
    rel = err.max() / (np.abs(expected).max() + 1e-12)
    print("max abs err:", err.max(), "rel:", rel)
    print("exec_time_ns:", res.exec_time_ns)
